# revision 18
# baseline (speedup 1.0000x reference)
"""Trainium2 Bass kernel for nn_GATRes (GATv2 x4 + dense per-graph attention).

Self-contained: kernel(**inputs) takes full inputs, shards 128 graphs/core
across 8 NeuronCores (data-parallel over graphs), runs the Bass/Tile kernel
via run_bass_kernel_spmd, and gathers the full [65536, 128] fp32 output.

v3 design notes:
- lrelu_0.2(s) = 0.8*relu(s) + 0.2*s. The logit att.lrelu(s) splits into
  0.8*att.relu(s) (PE: attP rows over relu'd messages, DMA-xbar-transposed
  fp16 to edge-major) + 0.2*att.s (linear: al[src]+ar[dst]+ae via tiny
  gather matmuls on host-precomputed Wl@att / Wr@att / ea@(We@att)).
- single [128,16] ACT exp per pair-layer; ACT stays on one table
  (Copy/Relu/Exp) to avoid ACT_TABLE_LOAD thrash.
- chunk of 8 pairs processed layer-phase-batched so each engine sees long
  uniform instruction runs and DMA-transpose latency is hidden.
"""
import sys
for _p in ("/opt/trn_rl_repo", "/root/.axon_site/_ro/trn_rl_repo"):
    if _p not in sys.path:
        sys.path.append(_p)
import numpy as np
import ml_dtypes

import concourse.bass as bass
import concourse.bacc as bacc
import concourse.tile as tile
from concourse import mybir
from concourse.masks import make_identity

F32 = mybir.dt.float32
F16 = mybir.dt.float16
BF16 = mybir.dt.bfloat16
AF = mybir.ActivationFunctionType
ALU = mybir.AluOpType
BF = ml_dtypes.bfloat16

B, S, EPG = 1024, 64, 256
N, E, D, H = B * S, B * EPG, 128, 4
NEG = 0.2
DEN_EPS = 1e-30


# ---------------------------------------------------------------- host prep

def host_prep(inputs, n_cores=8):
    """Build per-core input maps (numpy). All arrays bf16 except outputs."""
    x = np.asarray(inputs["x"], np.float32)
    ei = np.asarray(inputs["edge_index"])
    ea = np.asarray(inputs["edge_attr"], np.float32)
    src, dst = ei[0].astype(np.int64), ei[1].astype(np.int64)

    g_of_edge = dst // S
    assert np.array_equal(g_of_edge, src // S), "edges must be intra-graph"
    order = np.argsort(g_of_edge, kind="stable")
    if not np.array_equal(order, np.arange(E)):
        src, dst, ea = src[order], dst[order], ea[order]
    counts = np.bincount(g_of_edge, minlength=B)
    assert (counts == EPG).all(), "expect equal edges per graph"

    npair = B // 2
    pair_of_edge = np.arange(E) // (2 * EPG)
    e_in_pair = np.arange(E) % (2 * EPG)
    src_ip = (src - pair_of_edge * 2 * S).astype(np.int64)
    dst_ip = (dst - pair_of_edge * 2 * S).astype(np.int64)
    assert src_ip.min() >= 0 and src_ip.max() < 2 * S
    assert dst_ip.min() >= 0 and dst_ip.max() < 2 * S

    GsT = np.zeros((npair, 2 * S, 512), BF)     # [p, 128 n, 512 e]
    GdT = np.zeros((npair, 2 * S, 512), BF)
    Gdblk = np.zeros((npair, 128, 512), BF)     # [p, 128 ep, 4eb*128 n]
    GsT[pair_of_edge, src_ip, e_in_pair] = 1
    GdT[pair_of_edge, dst_ip, e_in_pair] = 1
    eb = e_in_pair // 128
    ep = e_in_pair % 128
    Gdblk[pair_of_edge, ep, eb * 128 + dst_ip] = 1

    xT = np.ascontiguousarray(x.T).astype(BF)   # [9, N]
    eaT = np.ascontiguousarray(ea.T).astype(BF)  # [3, E]

    g0_att = np.asarray(inputs["g0_att"], np.float32).reshape(D)     # [128]
    gWl = np.asarray(inputs["gWl"], np.float32)
    gWr = np.asarray(inputs["gWr"], np.float32)
    gWe = np.asarray(inputs["gWe"], np.float32)
    gatt = np.asarray(inputs["gatt"], np.float32)                    # [3,H,D]
    g0_Wl = np.asarray(inputs["g0_Wl"], np.float32)
    g0_Wr = np.asarray(inputs["g0_Wr"], np.float32)
    g0_We = np.asarray(inputs["g0_We"], np.float32)

    w = {}
    w["Wlr0"] = np.concatenate([g0_Wl, g0_Wr], 1).astype(BF)         # [9,256]
    w["We0"] = g0_We.astype(BF)                                      # [3,128]
    # attP{li} [128,16]: col h = 0.8 * att_h (cols H..15 zero)
    a0 = np.zeros((D, 16), np.float32)
    a0[:, 0] = 0.8 * g0_att
    w["attP0"] = a0.astype(BF)
    # Wlra{li} [*, 2H]: cols 0:H = Wl @ att_h, cols H:2H = Wr @ att_h
    wl0 = np.zeros((9, 16), np.float32)
    wl0[:, 0] = g0_Wl @ g0_att
    wl0[:, 4] = g0_Wr @ g0_att
    w["Wlra0"] = wl0.astype(BF)                                      # [9,16]
    for i in range(3):
        w[f"Wlr{i+1}"] = np.concatenate([gWl[i], gWr[i]], 1).astype(BF)  # [128,1024]
        w[f"We{i+1}"] = gWe[i].astype(BF)                                 # [3,512]
        ai = np.zeros((D, 16), np.float32)
        ai[:, 0:H] = 0.8 * gatt[i].T
        w[f"attP{i+1}"] = ai.astype(BF)
        wli = np.zeros((D, 16), np.float32)
        for h in range(H):
            wli[:, h] = gWl[i][:, h * D:(h + 1) * D] @ gatt[i][h]
            wli[:, 4 + h] = gWr[i][:, h * D:(h + 1) * D] @ gatt[i][h]
        w[f"Wlra{i+1}"] = wli.astype(BF)                                  # [128,16]
    w["Wqkv"] = np.concatenate(
        [inputs["Wq"], inputs["Wk"], inputs["Wv"]], 1).astype(np.float32).astype(BF)
    w["Wo_half"] = (np.asarray(inputs["Wo"], np.float32) * 0.5).astype(BF)

    # ae02: per pair per layer, 0.2 * (ea @ (We @ att)) edge-major.
    # layout [npair, 128 ep, 52]: cols 0:4 = L0 (eb), 4+16*i + eb*4+h = L1..3
    ae_all = np.zeros((npair, 128, 52), np.float32)
    we_att0 = g0_We @ g0_att                                  # [3]
    ae0 = 0.2 * (ea @ we_att0)                                # [E]
    ae_all[pair_of_edge, ep, eb] = ae0
    for i in range(3):
        WeAtt = np.einsum('chd,hd->ch', gWe[i].reshape(3, H, D), gatt[i])
        aei = 0.2 * (ea @ WeAtt)                              # [E, H]
        for h in range(H):
            ae_all[pair_of_edge, ep, 4 + 16 * i + eb * 4 + h] = aei[:, h]
    ae_all = ae_all.astype(BF)

    flags = {
        "blr0": not (np.any(inputs["g0_bl"]) or np.any(inputs["g0_br"])),
        "bias0": not np.any(inputs["g0_bias"]),
        "blr": not (np.any(inputs["gbl"]) or np.any(inputs["gbr"])),
        "gbias": not np.any(inputs["gbias"]),
        "bqkv": not (np.any(inputs["bq"]) or np.any(inputs["bk"]) or np.any(inputs["bv"])),
        "bo": not np.any(inputs["bo"]),
    }
    assert all(flags.values()), f"nonzero biases not supported in this build: {flags}"

    n_core = N // n_cores
    p_core = npair // n_cores
    in_maps = []
    for c in range(n_cores):
        m = dict(w)
        m["xT"] = np.ascontiguousarray(xT[:, c * n_core:(c + 1) * n_core])
        m["eaT"] = np.ascontiguousarray(
            eaT[:, c * p_core * 512:(c + 1) * p_core * 512])
        m["GsT"] = GsT[c * p_core:(c + 1) * p_core].reshape(p_core * 128, 512)
        m["GdT"] = GdT[c * p_core:(c + 1) * p_core].reshape(p_core * 128, 512)
        m["Gdblk"] = Gdblk[c * p_core:(c + 1) * p_core].reshape(p_core * 128, 512)
        m["aeALL"] = ae_all[c * p_core:(c + 1) * p_core].reshape(p_core * 128, 52)
        in_maps.append(m)
    return in_maps


# ---------------------------------------------------------------- emitter

def build_kernel(npairs=64):
    nc = bacc.Bacc()
    n_loc = npairs * 128

    d_xT = nc.declare_dram_parameter("xT", [9, n_loc], BF16, isOutput=False)
    d_eaT = nc.declare_dram_parameter("eaT", [3, npairs * 512], BF16, isOutput=False)
    d_GsT = nc.declare_dram_parameter("GsT", [n_loc, 512], BF16, isOutput=False)
    d_GdT = nc.declare_dram_parameter("GdT", [n_loc, 512], BF16, isOutput=False)
    d_Gdblk = nc.declare_dram_parameter("Gdblk", [n_loc, 512], BF16, isOutput=False)
    d_ae = nc.declare_dram_parameter("aeALL", [n_loc, 52], BF16, isOutput=False)
    d_w = {}
    for nm, shp in [("Wlr0", [9, 256]), ("We0", [3, 128]), ("attP0", [128, 16]),
                    ("Wlra0", [9, 16]),
                    ("Wlr1", [128, 1024]), ("We1", [3, 512]), ("attP1", [128, 16]),
                    ("Wlra1", [128, 16]),
                    ("Wlr2", [128, 1024]), ("We2", [3, 512]), ("attP2", [128, 16]),
                    ("Wlra2", [128, 16]),
                    ("Wlr3", [128, 1024]), ("We3", [3, 512]), ("attP3", [128, 16]),
                    ("Wlra3", [128, 16]),
                    ("Wqkv", [128, 384]), ("Wo_half", [128, 128])]:
        d_w[nm] = nc.declare_dram_parameter(nm, shp, BF16, isOutput=False)
    d_out = nc.declare_dram_parameter("out", [n_loc, 128], F32, isOutput=True)

    with tile.TileContext(nc) as tc:
        _emit(nc, tc, npairs, d_xT, d_eaT, d_GsT, d_GdT, d_Gdblk, d_ae, d_w, d_out)
    nc.finalize()
    return nc


def _emit(nc, tc, npairs, d_xT, d_eaT, d_GsT, d_GdT, d_Gdblk, d_ae, d_w, d_out):
    import contextlib
    ctx = contextlib.ExitStack()
    const = ctx.enter_context(tc.tile_pool(name="const", bufs=1))
    struct = ctx.enter_context(tc.tile_pool(name="struct", bufs=3))
    node = ctx.enter_context(tc.tile_pool(name="node", bufs=9))
    edge = ctx.enter_context(tc.tile_pool(name="edge", bufs=9))
    small = ctx.enter_context(tc.tile_pool(name="small", bufs=5))
    ps = ctx.enter_context(tc.tile_pool(name="ps", bufs=1, space="PSUM"))

    w = {}
    for nm, d in d_w.items():
        w[nm] = const.tile(list(d.shape), BF16, tag=f"w_{nm}", name=f"w_{nm}")
        nc.sync.dma_start(out=w[nm][:], in_=d.ap())
    xT = const.tile([9, npairs * 128], BF16, tag="xT")
    nc.sync.dma_start(out=xT[:], in_=d_xT.ap())
    idf = const.tile([128, 128], F32, tag="idf")
    make_identity(nc, idf[:])
    idb = const.tile([128, 128], BF16, tag="idb")
    make_identity(nc, idb[:])
    eps_t = const.tile([128, 4], F32, tag="eps")
    nc.vector.memset(eps_t[:], DEN_EPS)
    ones_b = const.tile([128, 1], BF16, tag="ones")
    nc.vector.memset(ones_b[:], 1.0)

    # ---------------- per-pair-layer phases; st = per-pair state dict
    def phase_node(li, p, st):
        """transpose+relu input, xl/xr node features, alr = [al|ar]."""
        heads = 1 if li == 0 else 4
        if li == 0:
            xlr_ps = ps.tile([128, 256], F32, tag="xs", bufs=2)
            nc.tensor.matmul(xlr_ps[:], xT[:, p * 128:(p + 1) * 128], w["Wlr0"][:],
                             start=True, stop=True)
            xlr = node.tile([128, 256], BF16, tag="xlr0", bufs=10)
            nc.scalar.activation(xlr[:], xlr_ps[:], AF.Copy)
            st["xl"], st["xr"] = xlr[:, 0:128], xlr[:, 128:256]
            alr_ps = ps.tile([128, 16], F32, tag="lg", bufs=2)
            nc.tensor.matmul(alr_ps[:], xT[:, p * 128:(p + 1) * 128], w["Wlra0"][:],
                             start=True, stop=True)
        else:
            h = st["h"]
            hT_ps = ps.tile([128, 128], F32, tag="mt", bufs=2)
            nc.tensor.transpose(hT_ps[:], h[:], idf[:])
            ghT = node.tile([128, 128], BF16, tag="ghT", bufs=10)
            nc.vector.tensor_scalar_max(ghT[:], in0=hT_ps[:], scalar1=0.0)
            st["ghT"] = ghT
            Wlr = w[f"Wlr{li}"]
            xl_ps = ps.tile([128, 512], F32, tag="xs", bufs=2)
            nc.tensor.matmul(xl_ps[:], ghT[:], Wlr[:, 0:512], start=True, stop=True)
            xl = node.tile([128, 512], BF16, tag="xl", bufs=10)
            nc.scalar.activation(xl[:], xl_ps[:], AF.Copy)
            xr_ps = ps.tile([128, 512], F32, tag="xs", bufs=2)
            nc.tensor.matmul(xr_ps[:], ghT[:], Wlr[:, 512:1024], start=True, stop=True)
            xr = node.tile([128, 512], BF16, tag="xr", bufs=10)
            nc.vector.tensor_scalar_mul(xr[:], in0=xr_ps[:], scalar1=1.0)
            st["xl"], st["xr"] = xl[:], xr[:]
            alr_ps = ps.tile([128, 16], F32, tag="lg", bufs=2)
            nc.tensor.matmul(alr_ps[:], ghT[:], w[f"Wlra{li}"][:],
                             start=True, stop=True)
        alr = small.tile([128, 16], BF16, tag="alr", bufs=10)
        nc.vector.tensor_scalar_mul(alr[:], in0=alr_ps[:], scalar1=1.0)
        st["alr"] = alr

    def phase_msg(li, p, st):
        """relu'd messages rB per head + 0.8*att rows -> fp16 -> DMA transpose."""
        heads = 1 if li == 0 else 4
        We = w[f"We{li}"]
        attP = w[f"attP{li}"]
        lgBT = edge.tile([128, 64 * heads], F16, tag="lgBT", bufs=10,
                         name=f"lgBT_{li}_{p}")
        for hh in range(heads):
            mt_ps = ps.tile([128, 512], F32, tag="mt", bufs=2)
            nc.tensor.matmul(mt_ps[:], st["xl"][:, hh * 128:(hh + 1) * 128],
                             st["GsT"][:], start=True, stop=False)
            nc.tensor.matmul(mt_ps[:], st["xr"][:, hh * 128:(hh + 1) * 128],
                             st["GdT"][:], start=False, stop=False)
            nc.tensor.matmul(mt_ps[:], We[:, hh * 128:(hh + 1) * 128],
                             st["eaT"][:], start=False, stop=True)
            rB = edge.tile([128, 512], BF16, tag="rB", bufs=34, name=f"rB{hh}_{p}")
            if hh < 3:
                nc.scalar.activation(rB[:], mt_ps[:], AF.Relu)
            else:
                nc.vector.tensor_scalar_max(rB[:], in0=mt_ps[:], scalar1=0.0)
            lgB_ps = ps.tile([16, 512], F32, tag="lg", bufs=2)
            nc.tensor.matmul(lgB_ps[:], attP[:], rB[:], start=True, stop=True)
            lgr = edge.tile([16, 512], F16, tag="lgr", bufs=4, name=f"lgr{hh}_{p}")
            if hh % 2 == 0:
                nc.scalar.activation(lgr[:], lgB_ps[:], AF.Copy)
            else:
                nc.vector.tensor_scalar_mul(lgr[:], in0=lgB_ps[:], scalar1=1.0)
            nc.sync.dma_start_transpose(
                out=lgBT[:, hh * 64:(hh + 1) * 64].rearrange("p (a b) -> p a b", b=16),
                in_=lgr[:])
        st["lgBT"] = lgBT

    def phase_soft(li, p, st):
        """P1 = 0.2*(al[src]+ar[dst]) (+0.2*ae const), add lgBT rows, exp, den."""
        heads = 1 if li == 0 else 4
        alr = st["alr"]
        P1_ps = ps.tile([128, 4 * heads], F32, tag="tiny", bufs=1,
                        name=f"P1_{li}_{p}")
        for ebi in range(4):
            nc.tensor.matmul(P1_ps[:, ebi * heads:(ebi + 1) * heads],
                             st["GsT"][:, ebi * 128:(ebi + 1) * 128],
                             alr[:, 0:heads], start=True, stop=False)
            nc.tensor.matmul(P1_ps[:, ebi * heads:(ebi + 1) * heads],
                             st["GdT"][:, ebi * 128:(ebi + 1) * 128],
                             alr[:, 4:4 + heads], start=False, stop=True)
        ae = st["ae"]
        ae_sl = ae[:, 0:4] if li == 0 else ae[:, 4 + 16 * (li - 1):4 + 16 * li]
        lg = small.tile([128, 4 * heads], F16, tag="lg_em", bufs=10,
                        name=f"lgem_{li}_{p}")
        # lg = (P1*0.2 + ae) first, then += lgBT strided rows
        nc.vector.scalar_tensor_tensor(lg[:], P1_ps[:], NEG, ae_sl,
                                       op0=ALU.mult, op1=ALU.add)
        lgBT = st["lgBT"]
        ex = small.tile([128, 4 * heads], BF16, tag="ex", bufs=10,
                        name=f"ex_{li}_{p}")
        for ebi in range(4):
            v = lgBT[:, 16 * ebi + 0: 16 * ebi + 65 * (heads - 1) + 1: 65]
            nc.vector.scalar_tensor_tensor(
                lg[:, ebi * heads:(ebi + 1) * heads], v, 1.0,
                lg[:, ebi * heads:(ebi + 1) * heads],
                op0=ALU.mult, op1=ALU.add)
        nc.scalar.activation(ex[:], lg[:], AF.Exp)
        st["ex"] = ex

        den_ps = ps.tile([128, heads], F32, tag="tiny", bufs=1,
                         name=f"den_{li}_{p}")
        for ebi in range(4):
            nc.tensor.matmul(den_ps[:], st["Gdblk"][:, ebi * 128:(ebi + 1) * 128],
                             ex[:, ebi * heads:(ebi + 1) * heads],
                             start=(ebi == 0), stop=(ebi == 3))
        denc = small.tile([128, heads], F32, tag="denc", bufs=10,
                          name=f"denc_{li}_{p}")
        scale8 = 1.0 if li == 0 else 8.0
        nc.vector.scalar_tensor_tensor(denc[:], den_ps[:], scale8, eps_t[:, 0:heads],
                                       op0=ALU.mult, op1=ALU.max)
        rden = small.tile([128, heads], F32, tag="rden", bufs=10,
                          name=f"rden_{li}_{p}")
        nc.vector.reciprocal(rden[:], denc[:])
        st["rden"] = rden

    def phase_out(li, p, st):
        """xs gather, val = ex*xs, scatter, normalize + merge + residual."""
        heads = 1 if li == 0 else 4
        fw = 128 * heads
        ex = st["ex"]
        out_ps = ps.tile([128, fw], F32, tag="out", bufs=1, name=f"out_{li}_{p}")
        for ebi in range(4):
            xs_ps = ps.tile([128, fw], F32, tag="xs", bufs=2, name=f"xs_{li}_{p}_{ebi}")
            nc.tensor.matmul(xs_ps[:], st["GsT"][:, ebi * 128:(ebi + 1) * 128],
                             st["xl"], start=True, stop=True)
            val = edge.tile([128, fw], BF16, tag="val", bufs=6,
                            name=f"val_{li}_{p}_{ebi}")
            exs = ex[:, ebi * heads:(ebi + 1) * heads].unsqueeze(-1) \
                .broadcast_to([128, heads, 128])
            nc.vector.tensor_tensor(
                val[:].rearrange("p (i j) -> p i j", i=heads),
                xs_ps[:].rearrange("p (i j) -> p i j", i=heads),
                exs, op=ALU.mult)
            nc.tensor.matmul(out_ps[:], st["Gdblk"][:, ebi * 128:(ebi + 1) * 128],
                             val[:], start=(ebi == 0), stop=(ebi == 3))
        rden = st["rden"]
        if li == 0:
            h_new = node.tile([128, 128], F32, tag="h", bufs=17, name=f"h_{li}_{p}")
            nc.scalar.activation(h_new[:], out_ps[:], AF.Copy, scale=rden[:, 0:1])
        else:
            t = node.tile([128, 128], F32, tag="t", bufs=4, name=f"t_{li}_{p}")
            nc.scalar.activation(t[:], out_ps[:, 0:128], AF.Copy, scale=rden[:, 0:1])
            for hh in range(1, 4):
                nc.vector.scalar_tensor_tensor(
                    t[:], out_ps[:, hh * 128:(hh + 1) * 128], rden[:, hh:hh + 1],
                    t[:], op0=ALU.mult, op1=ALU.add)
            h_new = node.tile([128, 128], F32, tag="h", bufs=17, name=f"h_{li}_{p}")
            nc.vector.scalar_tensor_tensor(h_new[:], st["h"][:], 0.5, t[:],
                                           op0=ALU.mult, op1=ALU.add)
        st["h"] = h_new

    def attn_final(p, h):
        """Dense per-graph attention + residual; returns fin fp32 [128,128]."""
        hT_ps = ps.tile([128, 128], F32, tag="mt", bufs=2)
        nc.tensor.transpose(hT_ps[:], h[:], idf[:])
        hfT = node.tile([128, 128], BF16, tag="hfT", bufs=4)
        nc.scalar.activation(hfT[:], hT_ps[:], AF.Copy)

        qT_ps = ps.tile([32, 512], F32, tag="lg", bufs=2)
        kT_ps = ps.tile([32, 512], F32, tag="lg", bufs=2)
        for hh in range(4):
            nc.tensor.matmul(qT_ps[:, hh * 128:(hh + 1) * 128],
                             w["Wqkv"][:, 32 * hh:32 * hh + 32], hfT[:],
                             start=True, stop=True)
            nc.tensor.matmul(kT_ps[:, hh * 128:(hh + 1) * 128],
                             w["Wqkv"][:, 128 + 32 * hh:128 + 32 * hh + 32], hfT[:],
                             start=True, stop=True)
        qT = node.tile([32, 512], BF16, tag="qT", bufs=4)
        nc.scalar.activation(qT[:], qT_ps[:], AF.Copy)
        kT = node.tile([32, 512], BF16, tag="kT", bufs=4)
        nc.vector.tensor_scalar_mul(kT[:], in0=kT_ps[:], scalar1=1.0)

        v_ps = ps.tile([128, 128], F32, tag="mt", bufs=2)
        nc.tensor.matmul(v_ps[:], hfT[:], w["Wqkv"][:, 256:384], start=True, stop=True)
        v01 = node.tile([64, 128], BF16, tag="v01", bufs=4)
        nc.scalar.activation(v01[:], v_ps[0:64, :], AF.Copy)
        v23 = node.tile([64, 128], BF16, tag="v23", bufs=4)
        nc.vector.tensor_scalar_mul(v23[:], in0=v_ps[64:128, :], scalar1=1.0)

        sc_ps = ps.tile([64, 512], F32, tag="out", bufs=1)
        for g in range(2):
            for hh in range(4):
                nc.tensor.matmul(
                    sc_ps[:, (g * 4 + hh) * 64:(g * 4 + hh + 1) * 64],
                    kT[:, hh * 128 + 64 * g:hh * 128 + 64 * g + 64],
                    qT[:, hh * 128 + 64 * g:hh * 128 + 64 * g + 64],
                    start=True, stop=True)
        expT = node.tile([64, 512], BF16, tag="expT", bufs=4)
        nc.scalar.activation(expT[:], sc_ps[:], AF.Exp, scale=float(1.0 / np.sqrt(32)))

        den_ps = ps.tile([64, 8], F32, tag="tiny", bufs=1)
        o_ps = ps.tile([64, 256], F32, tag="lg", bufs=2)
        for g in range(2):
            vg = v01[:] if g == 0 else v23[:]
            for hh in range(4):
                e_sl = expT[:, (g * 4 + hh) * 64:(g * 4 + hh + 1) * 64]
                nc.tensor.matmul(den_ps[:, g * 4 + hh:g * 4 + hh + 1],
                                 e_sl, ones_b[0:64, :], start=True, stop=True)
                nc.tensor.matmul(o_ps[:, g * 128 + hh * 32:g * 128 + (hh + 1) * 32],
                                 e_sl, vg[:, hh * 32:(hh + 1) * 32],
                                 start=True, stop=True)
        rden = small.tile([64, 8], F32, tag="rdena", bufs=4)
        nc.vector.reciprocal(rden[:], den_ps[:])
        o_sc = node.tile([64, 256], BF16, tag="o_sc", bufs=4)
        for gh in range(8):
            nc.vector.tensor_scalar_mul(o_sc[:, gh * 32:(gh + 1) * 32],
                                        in0=o_ps[:, gh * 32:(gh + 1) * 32],
                                        scalar1=rden[:, gh:gh + 1])
        oT_ps = ps.tile([128, 128], BF16, tag="tiny", bufs=1)
        for g in range(2):
            nc.tensor.transpose(oT_ps[:, g * 64:(g + 1) * 64],
                                o_sc[:, g * 128:(g + 1) * 128], idb[0:64, 0:64])
        oT = node.tile([128, 128], BF16, tag="oT", bufs=4)
        nc.scalar.activation(oT[:], oT_ps[:], AF.Copy)

        fin_ps = ps.tile([128, 128], F32, tag="xs", bufs=2)
        nc.tensor.matmul(fin_ps[:], oT[:], w["Wo_half"][:], start=True, stop=True)
        fin = node.tile([128, 128], F32, tag="fin", bufs=4)
        nc.vector.scalar_tensor_tensor(fin[:], h[:], 0.5, fin_ps[:],
                                       op0=ALU.mult, op1=ALU.add)
        return fin

    # ---------------- chunk loop, phase-batched
    CH = 8
    for p0 in range(0, npairs, CH):
        chunk = list(range(p0, min(p0 + CH, npairs)))
        sts = {}
        for p in chunk:
            st = {}
            GsT_t = struct.tile([128, 512], BF16, tag="GsT", bufs=10,
                                name=f"GsT_{p}")
            nc.sync.dma_start(out=GsT_t[:], in_=d_GsT.ap()[p * 128:(p + 1) * 128, :])
            GdT_t = struct.tile([128, 512], BF16, tag="GdT", bufs=10,
                                name=f"GdT_{p}")
            nc.sync.dma_start(out=GdT_t[:], in_=d_GdT.ap()[p * 128:(p + 1) * 128, :])
            Gdblk_t = struct.tile([128, 512], BF16, tag="Gdblk", bufs=10,
                                  name=f"Gdblk_{p}")
            nc.sync.dma_start(out=Gdblk_t[:],
                              in_=d_Gdblk.ap()[p * 128:(p + 1) * 128, :])
            ae_t = struct.tile([128, 52], BF16, tag="ae", bufs=10, name=f"ae_{p}")
            nc.sync.dma_start(out=ae_t[:], in_=d_ae.ap()[p * 128:(p + 1) * 128, :])
            eaT_t = struct.tile([3, 512], BF16, tag="eaT", bufs=10, name=f"eaT_{p}")
            nc.sync.dma_start(out=eaT_t[:], in_=d_eaT.ap()[:, p * 512:(p + 1) * 512])
            st["GsT"], st["GdT"], st["Gdblk"], st["ae"] = \
                GsT_t[:], GdT_t[:], Gdblk_t[:], ae_t[:]
            st["eaT"] = eaT_t[:]
            sts[p] = st
        for li in range(4):
            for p in chunk:
                phase_node(li, p, sts[p])
            for p in chunk:
                phase_msg(li, p, sts[p])
            for p in chunk:
                phase_soft(li, p, sts[p])
                phase_out(li, p, sts[p])
        for p in chunk:
            fin = attn_final(p, sts[p]["h"])
            nc.sync.dma_start(out=d_out.ap()[p * 128:(p + 1) * 128, :], in_=fin[:])

    ctx.close()


# ---------------------------------------------------------------- entry point

_CACHED_NC = None


def _get_nc():
    global _CACHED_NC
    if _CACHED_NC is None:
        _CACHED_NC = build_kernel(npairs=64)
    return _CACHED_NC


def kernel(**inputs):
    from concourse.bass_utils import run_bass_kernel_spmd
    in_maps = host_prep(inputs, n_cores=8)
    nc = _get_nc()
    res = run_bass_kernel_spmd(nc, in_maps, list(range(8)))
    return np.concatenate([res.results[c]["out"] for c in range(8)], axis=0)


# revision 19
# speedup vs baseline: 1.0683x; 1.0683x over previous
"""Trainium2 Bass kernel for nn_GATRes (GATv2 x4 + dense per-graph attention).

Self-contained: kernel(**inputs) takes full inputs, shards 128 graphs/core
across 8 NeuronCores (data-parallel over graphs), runs the Bass/Tile kernel
via run_bass_kernel_spmd, and gathers the full [65536, 128] fp32 output.

v3 design notes:
- lrelu_0.2(s) = 0.8*relu(s) + 0.2*s. The logit att.lrelu(s) splits into
  0.8*att.relu(s) (PE: attP rows over relu'd messages, DMA-xbar-transposed
  fp16 to edge-major) + 0.2*att.s (linear: al[src]+ar[dst]+ae via tiny
  gather matmuls on host-precomputed Wl@att / Wr@att / ea@(We@att)).
- single [128,16] ACT exp per pair-layer; ACT stays on one table
  (Copy/Relu/Exp) to avoid ACT_TABLE_LOAD thrash.
- chunk of 8 pairs processed layer-phase-batched so each engine sees long
  uniform instruction runs and DMA-transpose latency is hidden.
"""
import sys
for _p in ("/opt/trn_rl_repo", "/root/.axon_site/_ro/trn_rl_repo"):
    if _p not in sys.path:
        sys.path.append(_p)
import numpy as np
import ml_dtypes

import concourse.bass as bass
import concourse.bacc as bacc
import concourse.tile as tile
from concourse import mybir
from concourse.masks import make_identity

F32 = mybir.dt.float32
F16 = mybir.dt.float16
BF16 = mybir.dt.bfloat16
AF = mybir.ActivationFunctionType
ALU = mybir.AluOpType
BF = ml_dtypes.bfloat16

B, S, EPG = 1024, 64, 256
N, E, D, H = B * S, B * EPG, 128, 4
NEG = 0.2
DEN_EPS = 1e-30


# ---------------------------------------------------------------- host prep

def host_prep(inputs, n_cores=8):
    """Build per-core input maps (numpy). All arrays bf16 except outputs."""
    x = np.asarray(inputs["x"], np.float32)
    ei = np.asarray(inputs["edge_index"])
    ea = np.asarray(inputs["edge_attr"], np.float32)
    src, dst = ei[0].astype(np.int64), ei[1].astype(np.int64)

    g_of_edge = dst // S
    assert np.array_equal(g_of_edge, src // S), "edges must be intra-graph"
    order = np.argsort(g_of_edge, kind="stable")
    if not np.array_equal(order, np.arange(E)):
        src, dst, ea = src[order], dst[order], ea[order]
    counts = np.bincount(g_of_edge, minlength=B)
    assert (counts == EPG).all(), "expect equal edges per graph"

    npair = B // 2
    pair_of_edge = np.arange(E) // (2 * EPG)
    e_in_pair = np.arange(E) % (2 * EPG)
    src_ip = (src - pair_of_edge * 2 * S).astype(np.int64)
    dst_ip = (dst - pair_of_edge * 2 * S).astype(np.int64)
    assert src_ip.min() >= 0 and src_ip.max() < 2 * S
    assert dst_ip.min() >= 0 and dst_ip.max() < 2 * S

    GsT = np.zeros((npair, 2 * S, 512), BF)     # [p, 128 n, 512 e]
    GdT = np.zeros((npair, 2 * S, 512), BF)
    Gdblk = np.zeros((npair, 128, 512), BF)     # [p, 128 ep, 4eb*128 n]
    GsT[pair_of_edge, src_ip, e_in_pair] = 1
    GdT[pair_of_edge, dst_ip, e_in_pair] = 1
    eb = e_in_pair // 128
    ep = e_in_pair % 128
    Gdblk[pair_of_edge, ep, eb * 128 + dst_ip] = 1

    xT = np.ascontiguousarray(x.T).astype(BF)   # [9, N]
    eaT = np.ascontiguousarray(ea.T).astype(BF)  # [3, E]

    g0_att = np.asarray(inputs["g0_att"], np.float32).reshape(D)     # [128]
    gWl = np.asarray(inputs["gWl"], np.float32)
    gWr = np.asarray(inputs["gWr"], np.float32)
    gWe = np.asarray(inputs["gWe"], np.float32)
    gatt = np.asarray(inputs["gatt"], np.float32)                    # [3,H,D]
    g0_Wl = np.asarray(inputs["g0_Wl"], np.float32)
    g0_Wr = np.asarray(inputs["g0_Wr"], np.float32)
    g0_We = np.asarray(inputs["g0_We"], np.float32)

    w = {}
    w["Wlr0"] = np.concatenate([g0_Wl, g0_Wr], 1).astype(BF)         # [9,256]
    w["We0"] = g0_We.astype(BF)                                      # [3,128]
    # attP{li} [128,16]: col h = 0.8 * att_h (cols H..15 zero)
    a0 = np.zeros((D, 16), np.float32)
    a0[:, 0] = 0.8 * g0_att
    w["attP0"] = a0.astype(BF)
    # Wlra{li} [*, 2H]: cols 0:H = Wl @ att_h, cols H:2H = Wr @ att_h
    wl0 = np.zeros((9, 16), np.float32)
    wl0[:, 0] = g0_Wl @ g0_att
    wl0[:, 4] = g0_Wr @ g0_att
    w["Wlra0"] = wl0.astype(BF)                                      # [9,16]
    for i in range(3):
        w[f"Wlr{i+1}"] = np.concatenate([gWl[i], gWr[i]], 1).astype(BF)  # [128,1024]
        w[f"We{i+1}"] = gWe[i].astype(BF)                                 # [3,512]
        ai = np.zeros((D, 16), np.float32)
        ai[:, 0:H] = 0.8 * gatt[i].T
        w[f"attP{i+1}"] = ai.astype(BF)
        wli = np.zeros((D, 16), np.float32)
        for h in range(H):
            wli[:, h] = gWl[i][:, h * D:(h + 1) * D] @ gatt[i][h]
            wli[:, 4 + h] = gWr[i][:, h * D:(h + 1) * D] @ gatt[i][h]
        w[f"Wlra{i+1}"] = wli.astype(BF)                                  # [128,16]
    w["Wqkv"] = np.concatenate(
        [inputs["Wq"], inputs["Wk"], inputs["Wv"]], 1).astype(np.float32).astype(BF)
    w["Wo_half"] = (np.asarray(inputs["Wo"], np.float32) * 0.5).astype(BF)

    # ae02: per pair per layer, 0.2 * (ea @ (We @ att)) edge-major.
    # layout [npair, 128 ep, 52]: cols 0:4 = L0 (eb), 4+16*i + eb*4+h = L1..3
    ae_all = np.zeros((npair, 128, 52), np.float32)
    we_att0 = g0_We @ g0_att                                  # [3]
    ae0 = 0.2 * (ea @ we_att0)                                # [E]
    ae_all[pair_of_edge, ep, eb] = ae0
    for i in range(3):
        WeAtt = np.einsum('chd,hd->ch', gWe[i].reshape(3, H, D), gatt[i])
        aei = 0.2 * (ea @ WeAtt)                              # [E, H]
        for h in range(H):
            ae_all[pair_of_edge, ep, 4 + 16 * i + eb * 4 + h] = aei[:, h]
    ae_all = ae_all.astype(BF)

    flags = {
        "blr0": not (np.any(inputs["g0_bl"]) or np.any(inputs["g0_br"])),
        "bias0": not np.any(inputs["g0_bias"]),
        "blr": not (np.any(inputs["gbl"]) or np.any(inputs["gbr"])),
        "gbias": not np.any(inputs["gbias"]),
        "bqkv": not (np.any(inputs["bq"]) or np.any(inputs["bk"]) or np.any(inputs["bv"])),
        "bo": not np.any(inputs["bo"]),
    }
    assert all(flags.values()), f"nonzero biases not supported in this build: {flags}"

    n_core = N // n_cores
    p_core = npair // n_cores
    in_maps = []
    for c in range(n_cores):
        m = dict(w)
        m["xT"] = np.ascontiguousarray(xT[:, c * n_core:(c + 1) * n_core])
        m["eaT"] = np.ascontiguousarray(
            eaT[:, c * p_core * 512:(c + 1) * p_core * 512])
        m["GsT"] = GsT[c * p_core:(c + 1) * p_core].reshape(p_core * 128, 512)
        m["GdT"] = GdT[c * p_core:(c + 1) * p_core].reshape(p_core * 128, 512)
        m["Gdblk"] = Gdblk[c * p_core:(c + 1) * p_core].reshape(p_core * 128, 512)
        m["aeALL"] = ae_all[c * p_core:(c + 1) * p_core].reshape(p_core * 128, 52)
        in_maps.append(m)
    return in_maps


# ---------------------------------------------------------------- emitter

def build_kernel(npairs=64):
    nc = bacc.Bacc()
    n_loc = npairs * 128

    d_xT = nc.declare_dram_parameter("xT", [9, n_loc], BF16, isOutput=False)
    d_eaT = nc.declare_dram_parameter("eaT", [3, npairs * 512], BF16, isOutput=False)
    d_GsT = nc.declare_dram_parameter("GsT", [n_loc, 512], BF16, isOutput=False)
    d_GdT = nc.declare_dram_parameter("GdT", [n_loc, 512], BF16, isOutput=False)
    d_Gdblk = nc.declare_dram_parameter("Gdblk", [n_loc, 512], BF16, isOutput=False)
    d_ae = nc.declare_dram_parameter("aeALL", [n_loc, 52], BF16, isOutput=False)
    d_w = {}
    for nm, shp in [("Wlr0", [9, 256]), ("We0", [3, 128]), ("attP0", [128, 16]),
                    ("Wlra0", [9, 16]),
                    ("Wlr1", [128, 1024]), ("We1", [3, 512]), ("attP1", [128, 16]),
                    ("Wlra1", [128, 16]),
                    ("Wlr2", [128, 1024]), ("We2", [3, 512]), ("attP2", [128, 16]),
                    ("Wlra2", [128, 16]),
                    ("Wlr3", [128, 1024]), ("We3", [3, 512]), ("attP3", [128, 16]),
                    ("Wlra3", [128, 16]),
                    ("Wqkv", [128, 384]), ("Wo_half", [128, 128])]:
        d_w[nm] = nc.declare_dram_parameter(nm, shp, BF16, isOutput=False)
    d_out = nc.declare_dram_parameter("out", [n_loc, 128], F32, isOutput=True)

    with tile.TileContext(nc) as tc:
        _emit(nc, tc, npairs, d_xT, d_eaT, d_GsT, d_GdT, d_Gdblk, d_ae, d_w, d_out)
    nc.finalize()
    return nc


def _emit(nc, tc, npairs, d_xT, d_eaT, d_GsT, d_GdT, d_Gdblk, d_ae, d_w, d_out):
    import contextlib
    ctx = contextlib.ExitStack()
    const = ctx.enter_context(tc.tile_pool(name="const", bufs=1))
    struct = ctx.enter_context(tc.tile_pool(name="struct", bufs=3))
    node = ctx.enter_context(tc.tile_pool(name="node", bufs=9))
    edge = ctx.enter_context(tc.tile_pool(name="edge", bufs=9))
    small = ctx.enter_context(tc.tile_pool(name="small", bufs=5))
    ps = ctx.enter_context(tc.tile_pool(name="ps", bufs=1, space="PSUM"))

    w = {}
    for nm, d in d_w.items():
        w[nm] = const.tile(list(d.shape), BF16, tag=f"w_{nm}", name=f"w_{nm}")
        nc.sync.dma_start(out=w[nm][:], in_=d.ap())
    xT = const.tile([9, npairs * 128], BF16, tag="xT")
    nc.sync.dma_start(out=xT[:], in_=d_xT.ap())
    idf = const.tile([128, 128], F32, tag="idf")
    make_identity(nc, idf[:])
    idb = const.tile([128, 128], BF16, tag="idb")
    make_identity(nc, idb[:])
    eps_t = const.tile([128, 4], F32, tag="eps")
    nc.vector.memset(eps_t[:], DEN_EPS)
    ones_b = const.tile([128, 1], BF16, tag="ones")
    nc.vector.memset(ones_b[:], 1.0)

    # ---------------- per-pair-layer phases; st = per-pair state dict
    def phase_node_mm(li, p, st):
        """transpose (li>0); emitted for all pairs first to keep PE dense."""
        if li > 0:
            hT_ps = ps.tile([128, 128], F32, tag="mt", bufs=2, name=f"hT_{li}_{p}")
            nc.tensor.transpose(hT_ps[:], st["h"][:], idf[:])
            st["hT_ps"] = hT_ps

    def phase_node(li, p, st):
        """relu input, xl/xr node features, alr = [al|ar]."""
        if li == 0:
            xlr_ps = ps.tile([128, 256], F32, tag="xs", bufs=2)
            nc.tensor.matmul(xlr_ps[:], xT[:, p * 128:(p + 1) * 128], w["Wlr0"][:],
                             start=True, stop=True)
            xlr = node.tile([128, 256], BF16, tag="xlr0", bufs=10)
            nc.scalar.activation(xlr[:], xlr_ps[:], AF.Copy)
            st["xl"], st["xr"] = xlr[:, 0:128], xlr[:, 128:256]
            alr_ps = ps.tile([128, 16], F32, tag="lg", bufs=2)
            nc.tensor.matmul(alr_ps[:], xT[:, p * 128:(p + 1) * 128], w["Wlra0"][:],
                             start=True, stop=True)
        else:
            ghT = node.tile([128, 128], BF16, tag="ghT", bufs=10)
            nc.vector.tensor_scalar_max(ghT[:], in0=st.pop("hT_ps")[:], scalar1=0.0)
            st["ghT"] = ghT
            Wlr = w[f"Wlr{li}"]
            xl_ps = ps.tile([128, 512], F32, tag="xs", bufs=2)
            nc.tensor.matmul(xl_ps[:], ghT[:], Wlr[:, 0:512], start=True, stop=True)
            xl = node.tile([128, 512], BF16, tag="xl", bufs=10)
            nc.scalar.activation(xl[:], xl_ps[:], AF.Copy)
            xr_ps = ps.tile([128, 512], F32, tag="xs", bufs=2)
            nc.tensor.matmul(xr_ps[:], ghT[:], Wlr[:, 512:1024], start=True, stop=True)
            xr = node.tile([128, 512], BF16, tag="xr", bufs=10)
            nc.vector.tensor_scalar_mul(xr[:], in0=xr_ps[:], scalar1=1.0)
            st["xl"], st["xr"] = xl[:], xr[:]
            alr_ps = ps.tile([128, 16], F32, tag="lg", bufs=2)
            nc.tensor.matmul(alr_ps[:], ghT[:], w[f"Wlra{li}"][:],
                             start=True, stop=True)
        alr = small.tile([128, 16], BF16, tag="alr", bufs=10)
        nc.vector.tensor_scalar_mul(alr[:], in0=alr_ps[:], scalar1=1.0)
        st["alr"] = alr

    def phase_msg_quad(li, quad, sts):
        """messages rB for 4 pairs, att rows -> fp16 -> one fused DMA
        transpose per head. lgBT4 col(h, pq, eb, c) = h*256 + (4pq+eb)*16 + c;
        edge-major logit row part for head h sits at c == h."""
        heads = 1 if li == 0 else 4
        We = w[f"We{li}"]
        attP = w[f"attP{li}"]
        for p in quad:
            st = sts[p]
            st["rB"] = []
            for hh in range(heads):
                mt_ps = ps.tile([128, 512], F32, tag="mt", bufs=2,
                                name=f"mt_{li}_{p}_{hh}")
                nc.tensor.matmul(mt_ps[:], st["xl"][:, hh * 128:(hh + 1) * 128],
                                 st["GsT"][:], start=True, stop=False)
                nc.tensor.matmul(mt_ps[:], st["xr"][:, hh * 128:(hh + 1) * 128],
                                 st["GdT"][:], start=False, stop=False)
                nc.tensor.matmul(mt_ps[:], We[:, hh * 128:(hh + 1) * 128],
                                 st["eaT"][:], start=False, stop=True)
                rB = edge.tile([128, 512], BF16, tag="rB", bufs=34,
                               name=f"rB{hh}_{p}")
                if hh < 3:
                    nc.scalar.activation(rB[:], mt_ps[:], AF.Relu)
                else:
                    nc.vector.tensor_scalar_max(rB[:], in0=mt_ps[:], scalar1=0.0)
                st["rB"].append(rB)
        lgBT4 = edge.tile([128, 1024], F16, tag="lgBT", bufs=3,
                          name=f"lgBT_{li}_{quad[0]}")
        for hh in range(heads):
            lgr4 = edge.tile([16, 2048], F16, tag="lgr", bufs=5,
                             name=f"lgr{hh}_{quad[0]}")
            for i, p in enumerate(quad):
                lgB_ps = ps.tile([16, 512], F32, tag="lg", bufs=2,
                                 name=f"lgB_{li}_{p}_{hh}")
                nc.tensor.matmul(lgB_ps[:], attP[:], sts[p]["rB"][hh][:],
                                 start=True, stop=True)
                if (hh + i) % 2 == 0:
                    nc.scalar.activation(lgr4[:, i * 512:(i + 1) * 512], lgB_ps[:],
                                         AF.Copy)
                else:
                    nc.vector.tensor_scalar_mul(lgr4[:, i * 512:(i + 1) * 512],
                                                in0=lgB_ps[:], scalar1=1.0)
            nc.sync.dma_start_transpose(
                out=lgBT4[:, hh * 256:(hh + 1) * 256]
                    .rearrange("p (a b) -> p a b", b=16),
                in_=lgr4[:])
        for i, p in enumerate(quad):
            sts[p]["lgBT4"] = lgBT4
            sts[p]["pq"] = i
            del sts[p]["rB"]

    def phase_soft(li, p, st):
        """P1 = 0.2*(al[src]+ar[dst]) (+0.2*ae const), add lgBT rows, exp."""
        heads = 1 if li == 0 else 4
        alr = st["alr"]
        P1_ps = ps.tile([128, 4 * heads], F32, tag="tiny", bufs=1,
                        name=f"P1_{li}_{p}")
        for ebi in range(4):
            nc.tensor.matmul(P1_ps[:, ebi * heads:(ebi + 1) * heads],
                             st["GsT"][:, ebi * 128:(ebi + 1) * 128],
                             alr[:, 0:heads], start=True, stop=False)
            nc.tensor.matmul(P1_ps[:, ebi * heads:(ebi + 1) * heads],
                             st["GdT"][:, ebi * 128:(ebi + 1) * 128],
                             alr[:, 4:4 + heads], start=False, stop=True)
        ae = st["ae"]
        ae_sl = ae[:, 0:4] if li == 0 else ae[:, 4 + 16 * (li - 1):4 + 16 * li]
        lg = small.tile([128, 4 * heads], F16, tag="lg_em", bufs=10,
                        name=f"lgem_{li}_{p}")
        nc.vector.scalar_tensor_tensor(lg[:], P1_ps[:], NEG, ae_sl,
                                       op0=ALU.mult, op1=ALU.add)
        lgBT4 = st["lgBT4"]
        pq = st["pq"]
        ex = small.tile([128, 4 * heads], BF16, tag="ex", bufs=10,
                        name=f"ex_{li}_{p}")
        for ebi in range(4):
            c0 = 16 * (4 * pq + ebi)
            v = lgBT4[:, c0: c0 + 257 * (heads - 1) + 1: 257]
            nc.vector.scalar_tensor_tensor(
                lg[:, ebi * heads:(ebi + 1) * heads], v, 1.0,
                lg[:, ebi * heads:(ebi + 1) * heads],
                op0=ALU.mult, op1=ALU.add)
        nc.scalar.activation(ex[:], lg[:], AF.Exp)
        st["ex"] = ex

    def phase_out(li, p, st):
        """xs gather + den (shared stationaries), val = ex*xs, scatter,
        normalize + merge + residual. Scatter is emitted one eb behind the
        xs/den stream so the PE FIFO never waits on the DVE val op."""
        heads = 1 if li == 0 else 4
        fw = 128 * heads
        ex = st["ex"]
        out_ps = ps.tile([128, fw], F32, tag="out", bufs=1, name=f"out_{li}_{p}")
        den_ps = ps.tile([128, heads], F32, tag="mt", bufs=2, name=f"den_{li}_{p}")
        vals = []
        for ebi in range(4):
            xs_ps = ps.tile([128, fw], F32, tag="xs", bufs=2,
                            name=f"xs_{li}_{p}_{ebi}")
            nc.tensor.matmul(xs_ps[:], st["GsT"][:, ebi * 128:(ebi + 1) * 128],
                             st["xl"], start=True, stop=True)
            nc.tensor.matmul(den_ps[:], st["Gdblk"][:, ebi * 128:(ebi + 1) * 128],
                             ex[:, ebi * heads:(ebi + 1) * heads],
                             start=(ebi == 0), stop=(ebi == 3))
            val = edge.tile([128, fw], BF16, tag="val", bufs=6,
                            name=f"val_{li}_{p}_{ebi}")
            exs = ex[:, ebi * heads:(ebi + 1) * heads].unsqueeze(-1) \
                .broadcast_to([128, heads, 128])
            nc.vector.tensor_tensor(
                val[:].rearrange("p (i j) -> p i j", i=heads),
                xs_ps[:].rearrange("p (i j) -> p i j", i=heads),
                exs, op=ALU.mult)
            vals.append(val)
            if ebi >= 1:
                nc.tensor.matmul(out_ps[:],
                                 st["Gdblk"][:, (ebi - 1) * 128:ebi * 128],
                                 vals[ebi - 1][:],
                                 start=(ebi == 1), stop=False)
        nc.tensor.matmul(out_ps[:], st["Gdblk"][:, 384:512], vals[3][:],
                         start=False, stop=True)
        denc = small.tile([128, heads], F32, tag="denc", bufs=10,
                          name=f"denc_{li}_{p}")
        scale8 = 1.0 if li == 0 else 8.0
        nc.vector.scalar_tensor_tensor(denc[:], den_ps[:], scale8, eps_t[:, 0:heads],
                                       op0=ALU.mult, op1=ALU.max)
        rden = small.tile([128, heads], F32, tag="rden", bufs=10,
                          name=f"rden_{li}_{p}")
        nc.vector.reciprocal(rden[:], denc[:])
        if li == 0:
            h_new = node.tile([128, 128], F32, tag="h", bufs=17, name=f"h_{li}_{p}")
            nc.scalar.activation(h_new[:], out_ps[:], AF.Copy, scale=rden[:, 0:1])
        else:
            t = node.tile([128, 128], F32, tag="t", bufs=4, name=f"t_{li}_{p}")
            nc.scalar.activation(t[:], out_ps[:, 0:128], AF.Copy, scale=rden[:, 0:1])
            for hh in range(1, 4):
                nc.vector.scalar_tensor_tensor(
                    t[:], out_ps[:, hh * 128:(hh + 1) * 128], rden[:, hh:hh + 1],
                    t[:], op0=ALU.mult, op1=ALU.add)
            h_new = node.tile([128, 128], F32, tag="h", bufs=17, name=f"h_{li}_{p}")
            nc.vector.scalar_tensor_tensor(h_new[:], st["h"][:], 0.5, t[:],
                                           op0=ALU.mult, op1=ALU.add)
        st["h"] = h_new

    def attn_final(p, h):
        """Dense per-graph attention + residual; returns fin fp32 [128,128]."""
        hT_ps = ps.tile([128, 128], F32, tag="mt", bufs=2)
        nc.tensor.transpose(hT_ps[:], h[:], idf[:])
        hfT = node.tile([128, 128], BF16, tag="hfT", bufs=4)
        nc.scalar.activation(hfT[:], hT_ps[:], AF.Copy)

        qT_ps = ps.tile([32, 512], F32, tag="lg", bufs=2)
        kT_ps = ps.tile([32, 512], F32, tag="lg", bufs=2)
        for hh in range(4):
            nc.tensor.matmul(qT_ps[:, hh * 128:(hh + 1) * 128],
                             w["Wqkv"][:, 32 * hh:32 * hh + 32], hfT[:],
                             start=True, stop=True)
            nc.tensor.matmul(kT_ps[:, hh * 128:(hh + 1) * 128],
                             w["Wqkv"][:, 128 + 32 * hh:128 + 32 * hh + 32], hfT[:],
                             start=True, stop=True)
        qT = node.tile([32, 512], BF16, tag="qT", bufs=4)
        nc.scalar.activation(qT[:], qT_ps[:], AF.Copy)
        kT = node.tile([32, 512], BF16, tag="kT", bufs=4)
        nc.vector.tensor_scalar_mul(kT[:], in0=kT_ps[:], scalar1=1.0)

        v_ps = ps.tile([128, 128], F32, tag="mt", bufs=2)
        nc.tensor.matmul(v_ps[:], hfT[:], w["Wqkv"][:, 256:384], start=True, stop=True)
        v01 = node.tile([64, 128], BF16, tag="v01", bufs=4)
        nc.scalar.activation(v01[:], v_ps[0:64, :], AF.Copy)
        v23 = node.tile([64, 128], BF16, tag="v23", bufs=4)
        nc.vector.tensor_scalar_mul(v23[:], in0=v_ps[64:128, :], scalar1=1.0)

        sc_ps = ps.tile([64, 512], F32, tag="out", bufs=1)
        for g in range(2):
            for hh in range(4):
                nc.tensor.matmul(
                    sc_ps[:, (g * 4 + hh) * 64:(g * 4 + hh + 1) * 64],
                    kT[:, hh * 128 + 64 * g:hh * 128 + 64 * g + 64],
                    qT[:, hh * 128 + 64 * g:hh * 128 + 64 * g + 64],
                    start=True, stop=True)
        expT = node.tile([64, 512], BF16, tag="expT", bufs=4)
        nc.scalar.activation(expT[:], sc_ps[:], AF.Exp, scale=float(1.0 / np.sqrt(32)))

        den_ps = ps.tile([64, 8], F32, tag="tiny", bufs=1)
        o_ps = ps.tile([64, 256], F32, tag="lg", bufs=2)
        for g in range(2):
            vg = v01[:] if g == 0 else v23[:]
            for hh in range(4):
                e_sl = expT[:, (g * 4 + hh) * 64:(g * 4 + hh + 1) * 64]
                nc.tensor.matmul(den_ps[:, g * 4 + hh:g * 4 + hh + 1],
                                 e_sl, ones_b[0:64, :], start=True, stop=True)
                nc.tensor.matmul(o_ps[:, g * 128 + hh * 32:g * 128 + (hh + 1) * 32],
                                 e_sl, vg[:, hh * 32:(hh + 1) * 32],
                                 start=True, stop=True)
        rden = small.tile([64, 8], F32, tag="rdena", bufs=4)
        nc.vector.reciprocal(rden[:], den_ps[:])
        o_sc = node.tile([64, 256], BF16, tag="o_sc", bufs=4)
        for gh in range(8):
            nc.vector.tensor_scalar_mul(o_sc[:, gh * 32:(gh + 1) * 32],
                                        in0=o_ps[:, gh * 32:(gh + 1) * 32],
                                        scalar1=rden[:, gh:gh + 1])
        oT_ps = ps.tile([128, 128], BF16, tag="tiny", bufs=1)
        for g in range(2):
            nc.tensor.transpose(oT_ps[:, g * 64:(g + 1) * 64],
                                o_sc[:, g * 128:(g + 1) * 128], idb[0:64, 0:64])
        oT = node.tile([128, 128], BF16, tag="oT", bufs=4)
        nc.scalar.activation(oT[:], oT_ps[:], AF.Copy)

        fin_ps = ps.tile([128, 128], F32, tag="xs", bufs=2)
        nc.tensor.matmul(fin_ps[:], oT[:], w["Wo_half"][:], start=True, stop=True)
        fin = node.tile([128, 128], F32, tag="fin", bufs=4)
        nc.vector.scalar_tensor_tensor(fin[:], h[:], 0.5, fin_ps[:],
                                       op0=ALU.mult, op1=ALU.add)
        return fin

    # ---------------- chunk loop, phase-batched
    CH = 8
    for p0 in range(0, npairs, CH):
        chunk = list(range(p0, min(p0 + CH, npairs)))
        sts = {}
        for p in chunk:
            st = {}
            GsT_t = struct.tile([128, 512], BF16, tag="GsT", bufs=10,
                                name=f"GsT_{p}")
            nc.sync.dma_start(out=GsT_t[:], in_=d_GsT.ap()[p * 128:(p + 1) * 128, :])
            GdT_t = struct.tile([128, 512], BF16, tag="GdT", bufs=10,
                                name=f"GdT_{p}")
            nc.sync.dma_start(out=GdT_t[:], in_=d_GdT.ap()[p * 128:(p + 1) * 128, :])
            Gdblk_t = struct.tile([128, 512], BF16, tag="Gdblk", bufs=10,
                                  name=f"Gdblk_{p}")
            nc.sync.dma_start(out=Gdblk_t[:],
                              in_=d_Gdblk.ap()[p * 128:(p + 1) * 128, :])
            ae_t = struct.tile([128, 52], BF16, tag="ae", bufs=10, name=f"ae_{p}")
            nc.sync.dma_start(out=ae_t[:], in_=d_ae.ap()[p * 128:(p + 1) * 128, :])
            eaT_t = struct.tile([3, 512], BF16, tag="eaT", bufs=10, name=f"eaT_{p}")
            nc.sync.dma_start(out=eaT_t[:], in_=d_eaT.ap()[:, p * 512:(p + 1) * 512])
            st["GsT"], st["GdT"], st["Gdblk"], st["ae"] = \
                GsT_t[:], GdT_t[:], Gdblk_t[:], ae_t[:]
            st["eaT"] = eaT_t[:]
            sts[p] = st
        for li in range(4):
            for p in chunk:
                phase_node_mm(li, p, sts[p])
            for p in chunk:
                phase_node(li, p, sts[p])
            for quad in (chunk[0:4], chunk[4:8]):
                if quad:
                    phase_msg_quad(li, quad, sts)
            for p in chunk:
                phase_soft(li, p, sts[p])
            for p in chunk:
                phase_out(li, p, sts[p])
        for p in chunk:
            fin = attn_final(p, sts[p]["h"])
            nc.sync.dma_start(out=d_out.ap()[p * 128:(p + 1) * 128, :], in_=fin[:])

    ctx.close()


# ---------------------------------------------------------------- entry point

_CACHED_NC = None


def _get_nc():
    global _CACHED_NC
    if _CACHED_NC is None:
        _CACHED_NC = build_kernel(npairs=64)
    return _CACHED_NC


def kernel(**inputs):
    from concourse.bass_utils import run_bass_kernel_spmd
    in_maps = host_prep(inputs, n_cores=8)
    nc = _get_nc()
    res = run_bass_kernel_spmd(nc, in_maps, list(range(8)))
    return np.concatenate([res.results[c]["out"] for c in range(8)], axis=0)
